# revision 3
# baseline (speedup 1.0000x reference)
"""TRN2 Bass kernel for CompressedLinearLayer: out = x @ (A @ B.T).T + bias.

Computed low-rank: t = x @ B  (rank 512), out = t @ A.T + bias.
Sharding: data-parallel over the 8192 rows of x (1024 rows per core);
B, A.T, bias replicated. No collectives.

Device layouts (per core):
  xT   [4096, 1024]  x rows shard, transposed on host (d_in on partitions)
  b    [4096, 512]   B as-is (d_in on partitions)
  at   [512, 4096]   A.T (rank on partitions)
  bias [4096]
  out  [1024, 4096]  natural orientation

Per core the 1024 rows are processed in 2 pipelined blocks of 512:
  stage1(b): tT[r, m] = sum_k B[k, r] * xT[k, m]   (rank on partitions)
  stage2(b): out[m, d] = sum_r tT[r, m] * AT[r, d] + bias[d]
stage2(b0) PE work overlaps stage1(b1) input DMA; output stores go out on
the scalar HWDGE ring while inputs stream on the sync ring.

Matmuls run in float32r (TF32-like, full PE rate at N>=256, rel err ~1.5e-4).
"""
import numpy as np

import concourse.bacc as bacc
import concourse.mybir as mybir
import concourse.tile as tile
from concourse.bass_utils import run_bass_kernel_spmd

N_CORES = 8
BATCH, SEQ = 4, 2048
D_IN, D_OUT, RANK = 4096, 4096, 512
ROWS_TOTAL = BATCH * SEQ           # 8192
ROWS = ROWS_TOTAL // N_CORES       # 1024 rows per core

F32 = mybir.dt.float32
F32R = mybir.dt.float32r

KC = D_IN // 128     # 32 contraction chunks, stage 1
RC = RANK // 128     # 4 rank chunks
NBLK = 2             # row blocks per core
BROWS = ROWS // NBLK                 # 512 rows per block
MB2 = BROWS // 128   # 4 row chunks of 128 per block (stage-2 out partitions)
DB2 = D_OUT // 512   # 8 d_out blocks of 512 (stage-2 moving dim)

_compiled = {}


def _build():
    nc = bacc.Bacc("TRN2", target_bir_lowering=False, debug=False)

    xT_d = nc.declare_dram_parameter("xT", [D_IN, ROWS], F32R, isOutput=False)
    b_d = nc.declare_dram_parameter("b", [D_IN, RANK], F32R, isOutput=False)
    at_d = nc.declare_dram_parameter("at", [RANK, D_OUT], F32R, isOutput=False)
    bias_d = nc.declare_dram_parameter("bias", [D_OUT], F32, isOutput=False)
    out_d = nc.declare_dram_parameter("out", [ROWS, D_OUT], F32, isOutput=True)

    with tile.TileContext(nc) as tc:
        with (
            tc.tile_pool(name="wb", bufs=1) as wb,
            tc.tile_pool(name="xp", bufs=6) as xp,
            tc.tile_pool(name="tt", bufs=1) as ttp,
            tc.tile_pool(name="op", bufs=2) as op,
            tc.tile_pool(name="ps1", bufs=4, space="PSUM") as ps1p,
            tc.tile_pool(name="ps2", bufs=4, space="PSUM") as ps2p,
        ):
            # bias broadcast to all partitions: [128, 4096]
            bias_bc = wb.tile([128, D_OUT], F32, tag="bias_bc")
            nc.sync.dma_start(bias_bc[0:1, :], bias_d[None, :])
            nc.gpsimd.partition_broadcast(bias_bc[:], bias_bc[0:1, :])

            # B resident: 32 tiles [128, 512] (64KB/partition total)
            b_sb = [
                wb.tile([128, RANK], F32R, tag=f"b{k}", name=f"b{k}")
                for k in range(KC)
            ]
            # A.T resident: 4 tiles [128, 4096] (64KB/partition total)
            at_sb = [
                wb.tile([128, D_OUT], F32R, tag=f"at{r}", name=f"at{r}")
                for r in range(RC)
            ]
            # tT per block: 4 tiles [128, 512] f32r each
            tT = [
                [
                    ttp.tile([128, BROWS], F32R, tag=f"tT{b}_{r}", name=f"tT{b}_{r}")
                    for r in range(RC)
                ]
                for b in range(NBLK)
            ]

            def load_x(b, k):
                xk = xp.tile([128, BROWS], F32R, tag="xk", name=f"x{b}_{k}")
                nc.sync.dma_start(
                    xk[:], xT_d[k * 128:(k + 1) * 128, b * BROWS:(b + 1) * BROWS]
                )
                return xk

            def stage1(b):
                psum1 = [
                    ps1p.tile([128, BROWS], F32, tag="ps1", name=f"ps1_{b}_{i}")
                    for i in range(RC)
                ]
                for k in range(KC):
                    if b == 0:
                        # just-in-time interleave of B next to the x chunk that
                        # needs it on the sync DMA ring
                        nc.sync.dma_start(b_sb[k][:], b_d[k * 128:(k + 1) * 128, :])
                    xk = load_x(b, k)
                    for mc in range(RC):
                        nc.tensor.matmul(
                            psum1[mc][:],
                            b_sb[k][:, mc * 128:(mc + 1) * 128],
                            xk[:],
                            start=(k == 0),
                            stop=(k == KC - 1),
                        )
                for mc in range(RC):
                    nc.vector.tensor_copy(tT[b][mc][:], psum1[mc][:])

            def load_at():
                # after block-0 x; needed when stage2(0) starts
                for r in range(RC):
                    nc.sync.dma_start(at_sb[r][:], at_d[r * 128:(r + 1) * 128, :])

            def stage2(b):
                for rc2 in range(MB2):
                    row0 = rc2 * 128
                    for dch in range(DB2 // 4):     # two halves of d_out
                        psum2 = [
                            ps2p.tile(
                                [128, 512], F32, tag="ps2",
                                name=f"ps2_{b}_{rc2}_{dch}_{i}",
                            )
                            for i in range(4)
                        ]
                        for k in range(RC):
                            for dc in range(4):
                                d0 = (dch * 4 + dc) * 512
                                nc.tensor.matmul(
                                    psum2[dc][:],
                                    tT[b][k][:, row0:row0 + 128],
                                    at_sb[k][:, d0:d0 + 512],
                                    start=(k == 0),
                                    stop=(k == RC - 1),
                                )
                        ot = op.tile([128, 2048], F32, tag="ot", name=f"ot{b}_{rc2}_{dch}")
                        for dc in range(4):
                            d0 = (dch * 4 + dc) * 512
                            nc.vector.tensor_add(
                                ot[:, dc * 512:(dc + 1) * 512],
                                psum2[dc][:],
                                bias_bc[:, d0:d0 + 512],
                            )
                        # merged 1MB store on the scalar HWDGE ring
                        nc.scalar.dma_start(
                            out_d[
                                b * BROWS + row0:b * BROWS + row0 + 128,
                                dch * 2048:(dch + 1) * 2048,
                            ],
                            ot[:],
                        )

            stage1(0)
            load_at()
            stage2(0)
            stage1(1)
            stage2(1)

    nc.compile()
    return nc


def _get_nc():
    if "nc" not in _compiled:
        _compiled["nc"] = _build()
    return _compiled["nc"]


def run(inputs, trace=False, trace_kwargs=None):
    """Shard, execute on 8 cores, gather. Returns (output, BassKernelResults)."""
    x = np.asarray(inputs["x"], dtype=np.float32)
    A = np.asarray(inputs["A"], dtype=np.float32)
    B = np.asarray(inputs["B"], dtype=np.float32)
    bias = np.asarray(inputs["bias"], dtype=np.float32)

    x_flat = x.reshape(ROWS_TOTAL, D_IN)
    AT = np.ascontiguousarray(A.T)
    in_maps = []
    for i in range(N_CORES):
        xT_i = np.ascontiguousarray(x_flat[i * ROWS:(i + 1) * ROWS].T)
        in_maps.append({"xT": xT_i, "b": B, "at": AT, "bias": bias})

    nc = _get_nc()
    kwargs = {}
    if trace:
        kwargs["trace"] = True
        kwargs["trace_kwargs"] = trace_kwargs or {}
    res = run_bass_kernel_spmd(nc, in_maps, core_ids=list(range(N_CORES)), **kwargs)

    out = np.concatenate([res.results[i]["out"] for i in range(N_CORES)], axis=0)
    return out.reshape(BATCH, SEQ, D_OUT), res


def kernel(**inputs) -> np.ndarray:
    out, _ = run(inputs)
    return out
